# revision 1
# baseline (speedup 1.0000x reference)
"""Low-rank self-attention on 8 trn2 NeuronCores.

reference math (per batch b):
  q = x @ Wq.T            [S,R]
  k = x @ Wk.T            [S,R]
  v = x @ Wv.T            [S,D]
  P = softmax(q k^T / sqrt(R))    (mask is all-ones -> no-op)
  out = (P v) @ Wo.T      [S,D]

Sharding: 8 cores = (batch b in 0..3) x (query-half h in 0..1).
Each core computes attention for its 1024 query rows over the full 2048
keys of its batch. Host pre-transposes x and the weights so the kernel
needs no on-chip transposes:
  xt[i]  = x[b].T d-tile      [128d, 2048s]   (key cols permuted: own half first)
  wqt[i] = Wq.T d-tile        [128d, 128r]
  wvt[i] = Wv.T d-tile        [128d, 1024e]
On chip (all matmul operands bf16, PSUM accumulation f32):
  qT [128r, 1024q] ,  kT [128r, 2048k] ,  v[kt] [128k, 1024e]
  scoresT[k,q] = kT_chunk.T @ qT  -> exp (no max-subtract; scores bounded)
  s[q] = sum_k E[k,q] via tiny matmuls E.T @ ones  (accum PSUM [128q,1])
  ctxT[e,q] = sum_kt v[kt].T-block @ E[kt]  (accum PSUM)
  out[q,eo] = sum_et ctxT[et].T-block @ WoT[et] , then * (1/s[q]) per partition
softmax normalization is folded to the very end (it commutes with @ Wo.T).
"""

import math
import sys

import numpy as np

for _p in ("/opt/trn_rl_repo",):
    if _p not in sys.path:
        sys.path.append(_p)

import ml_dtypes  # noqa: E402

B, S, D, R = 4, 2048, 1024, 128
SQ = S // 2          # query rows per core
NCORES = 8
NDT = D // 128       # 8 d-tiles
NKT = S // 128       # 16 k-tiles
NQC = SQ // 512      # 2 q-chunks per core
SCALE = 1.0 / math.sqrt(R)

_CACHE = {}


def _build(dt_np):
    import concourse.bass as bass  # noqa: F401
    import concourse.tile as tile
    from concourse import bacc, mybir

    DT = mybir.dt.from_np(np.dtype(dt_np))
    F32 = mybir.dt.float32
    Exp = mybir.ActivationFunctionType.Exp

    nc = bacc.Bacc(
        "TRN2", target_bir_lowering=False, debug=False,
        enable_asserts=False, num_devices=NCORES,
    )
    xt_d = nc.dram_tensor("xt", [NDT, 128, S], DT, kind="ExternalInput").ap()
    wqt_d = nc.dram_tensor("wqt", [NDT, 128, R], DT, kind="ExternalInput").ap()
    wkt_d = nc.dram_tensor("wkt", [NDT, 128, R], DT, kind="ExternalInput").ap()
    wvt_d = nc.dram_tensor("wvt", [NDT, 128, D], DT, kind="ExternalInput").ap()
    wot_d = nc.dram_tensor("wot", [NDT, 128, D], DT, kind="ExternalInput").ap()
    out_d = nc.dram_tensor("out", [SQ, D], F32, kind="ExternalOutput").ap()

    from contextlib import ExitStack

    with tile.TileContext(nc) as tc, ExitStack() as es:
        pw = es.enter_context(tc.tile_pool(name="pw", bufs=1))
        px = es.enter_context(tc.tile_pool(name="px", bufs=1))
        pv = es.enter_context(tc.tile_pool(name="pv", bufs=1))
        pqk = es.enter_context(tc.tile_pool(name="pqk", bufs=1))
        pE = es.enter_context(tc.tile_pool(name="pE", bufs=NKT))
        pctx = es.enter_context(tc.tile_pool(name="pctx", bufs=8))
        posb = es.enter_context(tc.tile_pool(name="posb", bufs=3))
        prs = es.enter_context(tc.tile_pool(name="prs", bufs=2))
        ps_mm = es.enter_context(tc.tile_pool(name="ps_mm", bufs=3, space="PSUM"))
        ps_big = es.enter_context(tc.tile_pool(name="ps_big", bufs=4, space="PSUM"))
        ps_s = es.enter_context(tc.tile_pool(name="ps_s", bufs=1, space="PSUM"))

        mm = nc.tensor.matmul
        cp = nc.vector.tensor_copy

        # ---- persistent inputs -------------------------------------------
        wq = [pw.tile([128, R], DT, name=f"wq{i}") for i in range(NDT)]
        wk = [pw.tile([128, R], DT, name=f"wk{i}") for i in range(NDT)]
        wv = [pw.tile([128, D], DT, name=f"wv{i}") for i in range(NDT)]
        xts = [px.tile([128, S], DT, name=f"xt{i}") for i in range(NDT)]
        for i in range(NDT):
            nc.gpsimd.dma_start(out=wq[i], in_=wqt_d[i])
            nc.gpsimd.dma_start(out=wk[i], in_=wkt_d[i])
        # column-chunked so the first projection matmuls start after ~1MB;
        # wv interleaved early so v-proj isn't DMA-gated
        for c in range(2):
            for i in range(NDT):
                nc.sync.dma_start(out=xts[i][:, c * 512:(c + 1) * 512],
                                  in_=xt_d[i][:, c * 512:(c + 1) * 512])
        for i in range(NDT):
            nc.gpsimd.dma_start(out=wv[i], in_=wvt_d[i])
        for c in range(2, 4):
            for i in range(NDT):
                nc.sync.dma_start(out=xts[i][:, c * 512:(c + 1) * 512],
                                  in_=xt_d[i][:, c * 512:(c + 1) * 512])
        ones = pw.tile([128, 1], DT, name="ones")
        nc.vector.memset(ones, 1.0)

        qT = pqk.tile([128, SQ], DT, name="qT")
        kT = pqk.tile([128, S], DT, name="kT")
        vt = [pv.tile([128, D], DT, name=f"v{k}") for k in range(NKT)]

        # ---- phase A: projections ----------------------------------------
        for qc in range(NQC):
            ps = ps_mm.tile([128, 512], F32, name=f"q_ps{qc}", tag="mmps")
            for i in range(NDT):
                mm(ps, lhsT=wq[i], rhs=xts[i][:, qc * 512:(qc + 1) * 512],
                   start=(i == 0), stop=(i == NDT - 1))
            cp(qT[:, qc * 512:(qc + 1) * 512], ps)
        def kproj(kc):
            ps = ps_mm.tile([128, 512], F32, name=f"k_ps{kc}", tag="mmps")
            for i in range(NDT):
                mm(ps, lhsT=wk[i], rhs=xts[i][:, kc * 512:(kc + 1) * 512],
                   start=(i == 0), stop=(i == NDT - 1))
            cp(kT[:, kc * 512:(kc + 1) * 512], ps)

        def vproj(kt):
            for ec in range(2):
                ps = ps_big.tile([128, 512], F32, name=f"v_ps{kt}_{ec}", tag="bigps")
                for i in range(NDT):
                    mm(ps, lhsT=xts[i][:, kt * 128:(kt + 1) * 128],
                       rhs=wv[i][:, ec * 512:(ec + 1) * 512],
                       start=(i == 0), stop=(i == NDT - 1))
                cp(vt[kt][:, ec * 512:(ec + 1) * 512], ps)

        # consume in DMA-arrival order: xt chunks 0-1 land first, then wv,
        # then xt chunks 2-3 — so do k/v halves in that order.
        for kc in (0, 1):
            kproj(kc)
        for kt in range(NKT // 2):
            vproj(kt)
        for kc in (2, 3):
            kproj(kc)
        for kt in range(NKT // 2, NKT):
            vproj(kt)

        # wo arrives while phase A computes
        wo = [pw.tile([128, D], DT, name=f"wo{i}") for i in range(NDT)]
        for i in range(NDT):
            nc.gpsimd.dma_start(out=wo[i], in_=wot_d[i])

        # ---- phase B: attention per 512-wide q-chunk ---------------------
        for qc in range(NQC):
            qsl = qT[:, qc * 512:(qc + 1) * 512]
            s_ps = ps_s.tile([128, 4], F32, name=f"s_ps{qc}", tag="sps")
            Es = []
            # all score matmuls issue first so PE runs ahead of the exps
            for kt in range(NKT):
                sc = ps_mm.tile([128, 512], F32, name=f"sc{qc}_{kt}", tag="mmps")
                mm(sc, lhsT=kT[:, kt * 128:(kt + 1) * 128], rhs=qsl,
                   start=True, stop=True)
                Ek = pE.tile([128, 512], DT, name=f"E{qc}_{kt}", tag="E")
                nc.scalar.activation(Ek, sc, Exp, scale=SCALE)
                Es.append(Ek)
            ctxs = []
            for eh in range(2):
                cps = [ps_big.tile([128, 512], F32, name=f"c{qc}_{eh}_{j}", tag="bigps")
                       for j in range(4)]
                for kt in range(NKT):
                    for j in range(4):
                        e0 = eh * 512 + j * 128
                        mm(cps[j], lhsT=vt[kt][:, e0:e0 + 128], rhs=Es[kt],
                           start=(kt == 0), stop=(kt == NKT - 1))
                if eh == 0:
                    # rowsums here: all exps have landed by now, no PE stall.
                    # one accumulation group for the whole bank: start=True
                    # clears has_written for the entire bank, so only the very
                    # first mm may set it; later cols overwrite-then-accumulate.
                    for kt in range(NKT):
                        for j in range(4):
                            mm(s_ps[:, j:j + 1],
                               lhsT=Es[kt][:, j * 128:(j + 1) * 128],
                               rhs=ones, start=(kt == 0 and j == 0),
                               stop=(kt == NKT - 1 and j == 3))
                    rs = prs.tile([128, 4], F32, name=f"rs{qc}", tag="rs")
                    nc.vector.reciprocal(rs, s_ps)
                for j in range(4):
                    ct = pctx.tile([128, 512], DT, name=f"ct{qc}_{eh}_{j}", tag="ctx")
                    cp(ct, cps[j])
                    ctxs.append(ct)

            for qs in range(4):
                for eo in range(2):
                    ops = ps_mm.tile([128, 512], F32, name=f"o{qc}_{qs}_{eo}", tag="mmps")
                    for et in range(NDT):
                        mm(ops, lhsT=ctxs[et][:, qs * 128:(qs + 1) * 128],
                           rhs=wo[et][:, eo * 512:(eo + 1) * 512],
                           start=(et == 0), stop=(et == NDT - 1))
                    osb = posb.tile([128, 512], F32, name=f"osb{qc}_{qs}_{eo}", tag="osb")
                    nc.scalar.mul(osb, ops, rs[:, qs:qs + 1])
                    q0 = qc * 512 + qs * 128
                    nc.sync.dma_start(out=out_d[q0:q0 + 128, eo * 512:(eo + 1) * 512],
                                      in_=osb)

    nc.compile()
    return nc


def _prep_inputs(x, Wq, Wk, Wv, Wo, dt_np):
    """Host-side shard + transpose. Returns per-core input dicts."""
    def dtile(wT, n):  # [D, n] -> [NDT, 128, n]
        return np.ascontiguousarray(wT.reshape(NDT, 128, n).astype(dt_np))

    wqt = dtile(Wq.T, R)
    wkt = dtile(Wk.T, R)
    wvt = dtile(Wv.T, D)
    wot = dtile(Wo.T, D)
    in_maps = []
    for c in range(NCORES):
        b, h = divmod(c, 2)
        xb = x[b]
        # own query half first; k-order permutation is softmax/ctx-invariant
        xperm = np.concatenate([xb[h * SQ:(h + 1) * SQ], xb[(1 - h) * SQ:(2 - h) * SQ]], 0)
        xt = np.ascontiguousarray(xperm.T.reshape(NDT, 128, S).astype(dt_np))
        in_maps.append({"xt": xt, "wqt": wqt, "wkt": wkt, "wvt": wvt, "wot": wot})
    return in_maps


def _run(inputs, dt_np=ml_dtypes.bfloat16, trace=False, **kw):
    from concourse.bass_utils import run_bass_kernel_spmd

    key = np.dtype(dt_np).str
    if key not in _CACHE:
        _CACHE[key] = _build(dt_np)
    nc = _CACHE[key]
    in_maps = _prep_inputs(inputs["x"], inputs["Wq"], inputs["Wk"],
                           inputs["Wv"], inputs["Wo"], dt_np)
    res = run_bass_kernel_spmd(nc, in_maps, core_ids=list(range(NCORES)),
                               trace=trace, **kw)
    out = np.empty((B, S, D), np.float32)
    for c in range(NCORES):
        b, h = divmod(c, 2)
        out[b, h * SQ:(h + 1) * SQ] = res.results[c]["out"]
    return out, res


def kernel(x, mask, Wq, Wk, Wv, Wo):
    # mask is all-ones by construction (spec fill=ones) -> identity.
    out, _ = _run({"x": np.asarray(x, np.float32), "Wq": np.asarray(Wq, np.float32),
                   "Wk": np.asarray(Wk, np.float32), "Wv": np.asarray(Wv, np.float32),
                   "Wo": np.asarray(Wo, np.float32)})
    return out



# revision 7
# speedup vs baseline: 1.6044x; 1.6044x over previous
"""Low-rank self-attention on 8 trn2 NeuronCores.

reference math (per batch b):
  q = x @ Wq.T            [S,R]
  k = x @ Wk.T            [S,R]
  v = x @ Wv.T            [S,D]
  P = softmax(q k^T / sqrt(R))    (mask is all-ones -> no-op)
  out = (P v) @ Wo.T      [S,D]

Key restructuring: (P (x Wv^T)) Wo^T == (P x) (Wo Wv)^T.  The host
precomputes W' = Wo @ Wv once (weight-only), so the device computes
  ctx1 = E @ x    (E = exp(scores), unnormalized)
  out  = diag(1/rowsum(E)) @ (ctx1 @ W'^T)
which removes the entire v-projection (x is already on chip) — per-core
PE work drops from ~6.0G to ~3.9G MACs.

Sharding: 8 cores = (batch b in 0..3) x (query-half h in 0..1).
Each core computes attention for its 1024 query rows over the full 2048
keys of its batch.  Host ships x[b] in BOTH layouts (d-major for the
q/k projections, k-major for ctx1) so no on-chip transposes are needed:
  xt[p, i, s] = x[b].T d-tile i      [128d, 8, 2048s] (key cols permuted)
  xk[p, t, d] = x[b] k-tile t        [128k, 16, 1024d]
  wq/wk[p, i*128+r] = Wq.T d-tiles   [128d, 8*128r]
  wpt[p, i, e]     = W'.T d-tiles    [128d, 8, 1024e]
On chip (all matmul operands bf16, PSUM accumulation f32):
  qT [128r, 1024q], kT [128r, 2048k]
  scoresT[k,q] = kT_tile.T @ qT -> exp (no max-subtract; scores bounded)
  ctx1T[d,q]  = sum_kt xk[kt,dslice].T @ E[kt]   (accum PSUM, 8 d-tiles)
  s[q] = sum_k E[k,q] via tiny matmuls E.T @ ones
  out[q,e] = sum_dt ctx1T[dt].T-block @ W'T[dt], then * (1/s[q])

The PE instruction stream is hand-interleaved so the Tensor engine never
idles (the cost model halves PE speed for 3us after any idle gap):
score matmuls (which are Act-exp paced) are woven between k-projection
and ctx1 matmuls, and ctx1's first half chases the xk DMA kt-by-kt.
"""

import math
import sys

import numpy as np

for _p in ("/opt/trn_rl_repo",):
    if _p not in sys.path:
        sys.path.append(_p)

import ml_dtypes  # noqa: E402

B, S, D, R = 4, 2048, 1024, 128
SQ = S // 2          # query rows per core
NCORES = 8
NDT = D // 128       # 8 d-tiles
NKT = S // 128       # 16 k-tiles
NQC = SQ // 512      # 2 q-chunks per core
SCALE = 1.0 / math.sqrt(R)

_CACHE = {}


def _build(dt_np):
    import concourse.bass as bass  # noqa: F401
    import concourse.tile as tile
    from concourse import bacc, mybir

    DT = mybir.dt.from_np(np.dtype(dt_np))
    F32 = mybir.dt.float32
    Exp = mybir.ActivationFunctionType.Exp

    nc = bacc.Bacc(
        "TRN2", target_bir_lowering=False, debug=False,
        enable_asserts=False, num_devices=NCORES,
    )
    xt_d = nc.dram_tensor("xt", [128, NDT, S], DT, kind="ExternalInput").ap()
    xk_d = nc.dram_tensor("xk", [128, NKT, D], DT, kind="ExternalInput").ap()
    wq_d = nc.dram_tensor("wq", [128, NDT * R], DT, kind="ExternalInput").ap()
    wk_d = nc.dram_tensor("wk", [128, NDT * R], DT, kind="ExternalInput").ap()
    wpt_d = nc.dram_tensor("wpt", [128, NDT, D], DT, kind="ExternalInput").ap()
    out_d = nc.dram_tensor("out", [SQ, D], F32, kind="ExternalOutput").ap()

    from contextlib import ExitStack

    with tile.TileContext(nc) as tc, ExitStack() as es:
        pw = es.enter_context(tc.tile_pool(name="pw", bufs=1))
        px = es.enter_context(tc.tile_pool(name="px", bufs=1))
        pqk = es.enter_context(tc.tile_pool(name="pqk", bufs=1))
        pE = es.enter_context(tc.tile_pool(name="pE", bufs=NQC * NKT))
        pctx = es.enter_context(tc.tile_pool(name="pctx", bufs=16))
        posb = es.enter_context(tc.tile_pool(name="posb", bufs=4))
        prs = es.enter_context(tc.tile_pool(name="prs", bufs=2))
        ps_mm = es.enter_context(tc.tile_pool(name="ps_mm", bufs=3, space="PSUM"))
        ps_big = es.enter_context(tc.tile_pool(name="ps_big", bufs=4, space="PSUM"))

        mm = nc.tensor.matmul
        cp = nc.vector.tensor_copy

        # ---- persistent inputs, priority DMA order on the sync queue ------
        wq = pw.tile([128, NDT * R], DT, name="wq")
        wk = pw.tile([128, NDT * R], DT, name="wk")
        xts = px.tile([128, NDT, S], DT, name="xts")
        xks = px.tile([128, NKT, D], DT, name="xks")
        wpt = pw.tile([128, NDT, D], DT, name="wpt")
        nc.sync.dma_start(out=wq, in_=wq_d)
        for i in range(NDT):  # own-query half first: q-proj starts early
            nc.sync.dma_start(out=xts[:, i, 0:SQ], in_=xt_d[:, i, 0:SQ])
        nc.sync.dma_start(out=wk, in_=wk_d)
        for i in range(NDT):
            nc.sync.dma_start(out=xts[:, i, SQ:S], in_=xt_d[:, i, SQ:S])
        for t in range(NKT):  # staggered so ctx1 can chase arrival kt-by-kt
            nc.sync.dma_start(out=xks[:, t], in_=xk_d[:, t])
        nc.sync.dma_start(out=wpt, in_=wpt_d)
        ones = pw.tile([128, 1], DT, name="ones")
        nc.vector.memset(ones, 1.0)

        qT = pqk.tile([128, SQ], DT, name="qT")
        kT = pqk.tile([128, S], DT, name="kT")

        Es = [[None] * NKT for _ in range(NQC)]

        def emit_score(qc, kt):
            sc = ps_mm.tile([128, 512], F32, name=f"sc{qc}_{kt}", tag="mmps")
            mm(sc, lhsT=kT[:, kt * 128:(kt + 1) * 128],
               rhs=qT[:, qc * 512:(qc + 1) * 512], start=True, stop=True)
            Ek = pE.tile([128, 512], DT, name=f"E{qc}_{kt}", tag="E")
            nc.scalar.activation(Ek, sc, Exp, scale=SCALE)
            Es[qc][kt] = Ek

        # ---- phase A: q projection (own 512-col chunks) -------------------
        for qc in range(NQC):
            ps = ps_mm.tile([128, 512], F32, name=f"q_ps{qc}", tag="mmps")
            for i in range(NDT):
                mm(ps, lhsT=wq[:, i * R:(i + 1) * R],
                   rhs=xts[:, i, qc * 512:(qc + 1) * 512],
                   start=(i == 0), stop=(i == NDT - 1))
            cp(qT[:, qc * 512:(qc + 1) * 512], ps)

        # ---- phase B/C: k projection, scores qc0 kt0-3 woven into c2/c3 ---
        score_q = [(0, kt) for kt in range(NKT)] + [(1, kt) for kt in range(NKT)]

        def kproj(kc, scores_after):
            ps = ps_mm.tile([128, 512], F32, name=f"k_ps{kc}", tag="mmps")
            for i in range(NDT):
                mm(ps, lhsT=wk[:, i * R:(i + 1) * R],
                   rhs=xts[:, i, kc * 512:(kc + 1) * 512],
                   start=(i == 0), stop=(i == NDT - 1))
                if i in (3, 7) and scores_after:
                    emit_score(*score_q.pop(0))
            cp(kT[:, kc * 512:(kc + 1) * 512], ps)

        kproj(0, False)
        kproj(1, False)
        kproj(2, True)   # scores qc0 kt0,kt1 (kT c0/c1 ready)
        kproj(3, True)   # scores qc0 kt2,kt3

        # ---- ctx1 helpers -------------------------------------------------
        ctxs = [[None] * NDT for _ in range(NQC)]

        def ctx_bank(qc, j):
            return ps_big.tile([128, 512], F32, name=f"c{qc}_{j}", tag="bigps")

        def ctx_mm(bank, qc, j, kt):
            mm(bank, lhsT=xks[:, kt, j * 128:(j + 1) * 128], rhs=Es[qc][kt],
               start=(kt == 0), stop=(kt == NKT - 1))

        def ctx_finish(bank, qc, j):
            ct = pctx.tile([128, 512], DT, name=f"ct{qc}_{j}", tag="ctx")
            cp(ct, bank)
            ctxs[qc][j] = ct

        def ctx_group(qc, j):  # one d-tile, all 16 kt back-to-back
            bank = ctx_bank(qc, j)
            for kt in range(NKT):
                ctx_mm(bank, qc, j, kt)
            ctx_finish(bank, qc, j)

        def rowsum(qc):
            s_ps = ps_mm.tile([128, 4], F32, name=f"s_ps{qc}", tag="mmps")
            for kt in range(NKT):
                for j in range(4):
                    mm(s_ps[:, j:j + 1],
                       lhsT=Es[qc][kt][:, j * 128:(j + 1) * 128],
                       rhs=ones, start=(kt == 0 and j == 0),
                       stop=(kt == NKT - 1 and j == 3))
            rs = prs.tile([128, 4], F32, name=f"rs{qc}", tag="rs")
            nc.vector.reciprocal(rs, s_ps)
            return rs

        # ---- phase D: ctx1 qc0 d-tiles 0,1 chase the xk DMA; remaining ----
        # score matmuls (Act-exp paced) woven in so PE keeps issuing.
        b0, b1 = ctx_bank(0, 0), ctx_bank(0, 1)
        for kt in range(NKT):
            for _ in range(2):
                if score_q:
                    emit_score(*score_q.pop(0))
            ctx_mm(b0, 0, 0, kt)
            ctx_mm(b1, 0, 1, kt)
        ctx_finish(b0, 0, 0)
        ctx_finish(b1, 0, 1)
        # d-tiles 2..7 back-to-back (xk fully resident by now)
        ctx_group(0, 2)
        ctx_group(0, 3)
        ctx_group(0, 4)
        ctx_group(0, 5)
        rs0 = rowsum(0)
        ctx_group(0, 6)
        ctx_group(0, 7)

        # ---- out projection -----------------------------------------------
        def out_pair(qc, qs, eo, rs, et_order=None):
            ops = ps_mm.tile([128, 512], F32, name=f"o{qc}_{qs}_{eo}", tag="mmps")
            for n, et in enumerate(et_order or range(NDT)):
                mm(ops, lhsT=ctxs[qc][et][:, qs * 128:(qs + 1) * 128],
                   rhs=wpt[:, et, eo * 512:(eo + 1) * 512],
                   start=(n == 0), stop=(n == NDT - 1))
            osb = posb.tile([128, 512], F32, name=f"osb{qc}_{qs}_{eo}", tag="osb")
            nc.scalar.mul(osb, ops, rs[:, qs:qs + 1])
            q0 = qc * 512 + qs * 128
            nc.sync.dma_start(out=out_d[q0:q0 + 128, eo * 512:(eo + 1) * 512],
                              in_=osb)

        # ctx1 qc1 first half, then out qc0 (its ctx cps complete meanwhile)
        ctx_group(1, 0)
        ctx_group(1, 1)
        ctx_group(1, 2)
        ctx_group(1, 3)
        for qs in range(4):
            for eo in range(2):
                out_pair(0, qs, eo, rs0)
        ctx_group(1, 4)
        rs1 = rowsum(1)
        ctx_group(1, 5)
        ctx_group(1, 6)
        ctx_group(1, 7)
        out_pair(1, 0, 0, rs1)
        out_pair(1, 0, 1, rs1)
        for qs in range(1, 4):
            for eo in range(2):
                if qs == 3 and eo == 1:
                    continue
                out_pair(1, qs, eo, rs1)
        # last pair in two 256-col chunks: the tail after the final matmul is
        # a fixed mul->descgen->dma->semprop->drain chain, so keep the final
        # transfer short (but not so many chunks that HWDGE descgen, 625ns
        # per DMA instruction, serializes behind the last matmul instead)
        for ec in range(2):
            ops = ps_mm.tile([128, 256], F32, name=f"olast{ec}", tag="mmps")
            e0 = 512 + ec * 256
            for et in range(NDT):
                mm(ops, lhsT=ctxs[1][et][:, 3 * 128:4 * 128],
                   rhs=wpt[:, et, e0:e0 + 256],
                   start=(et == 0), stop=(et == NDT - 1))
            osb = posb.tile([128, 256], F32, name=f"osbl{ec}", tag="osbl")
            nc.scalar.mul(osb, ops, rs1[:, 3:4])
            nc.sync.dma_start(out=out_d[896:1024, e0:e0 + 256], in_=osb)

    nc.compile()
    return nc


def _prep_inputs(x, Wq, Wk, Wv, Wo, dt_np):
    """Host-side shard + transpose + weight fold. Returns per-core inputs."""
    Wp = (Wo.astype(np.float64) @ Wv.astype(np.float64)).astype(np.float32)

    def dtile(wT, n):  # [D, n] -> [128, NDT, n] (partition-major d-tiles)
        return np.ascontiguousarray(
            wT.reshape(NDT, 128, n).transpose(1, 0, 2).astype(dt_np))

    wq = np.ascontiguousarray(dtile(Wq.T, R).reshape(128, NDT * R))
    wk = np.ascontiguousarray(dtile(Wk.T, R).reshape(128, NDT * R))
    wpt = dtile(Wp.T, D)
    in_maps = []
    for c in range(NCORES):
        b, h = divmod(c, 2)
        xb = x[b]
        # own query half first; k-order permutation is softmax/ctx-invariant
        xperm = np.concatenate([xb[h * SQ:(h + 1) * SQ], xb[(1 - h) * SQ:(2 - h) * SQ]], 0)
        xt = np.ascontiguousarray(
            xperm.T.reshape(NDT, 128, S).transpose(1, 0, 2).astype(dt_np))
        xk = np.ascontiguousarray(
            xperm.reshape(NKT, 128, D).transpose(1, 0, 2).astype(dt_np))
        in_maps.append({"xt": xt, "xk": xk, "wq": wq, "wk": wk, "wpt": wpt})
    return in_maps


def _run(inputs, dt_np=ml_dtypes.bfloat16, trace=False, **kw):
    from concourse.bass_utils import run_bass_kernel_spmd

    key = np.dtype(dt_np).str
    if key not in _CACHE:
        _CACHE[key] = _build(dt_np)
    nc = _CACHE[key]
    in_maps = _prep_inputs(inputs["x"], inputs["Wq"], inputs["Wk"],
                           inputs["Wv"], inputs["Wo"], dt_np)
    res = run_bass_kernel_spmd(nc, in_maps, core_ids=list(range(NCORES)),
                               trace=trace, **kw)
    out = np.empty((B, S, D), np.float32)
    for c in range(NCORES):
        b, h = divmod(c, 2)
        out[b, h * SQ:(h + 1) * SQ] = res.results[c]["out"]
    return out, res


def kernel(x, mask, Wq, Wk, Wv, Wo):
    # mask is all-ones by construction (spec fill=ones) -> identity.
    out, _ = _run({"x": np.asarray(x, np.float32), "Wq": np.asarray(Wq, np.float32),
                   "Wk": np.asarray(Wk, np.float32), "Wv": np.asarray(Wv, np.float32),
                   "Wo": np.asarray(Wo, np.float32)})
    return out
